# revision 4
# baseline (speedup 1.0000x reference)
"""Multi-head attention Trainium2 kernel (8 NeuronCores).

Problem: x[2,2048,1024] -> MHA(16 heads, d=64) -> out[2,2048,1024], fp32.

Sharding: 2-way data parallel on batch x 4-way tensor parallel on heads.
Core c handles batch c//4 and heads 4*(c%4) .. 4*(c%4)+3 (a 256-wide slice
of the Wq/Wk/Wv columns and Wo rows). Each core returns a partial output
[2048,1024]; the host sums the 4 TP partials per batch and adds the bias
terms (bo, and bv@Wo which is separable because softmax rows sum to 1;
bk drops out of softmax entirely since (q+bq)@bk is constant along keys).

On-core dataflow. Projections and the output matmul run in fp32r (e8m11
operands via host pre-round, fp32 accumulate, 1 cycle/row on the PE);
the S = K^T Q matmul runs in fp8e4m3 DoubleRow perf mode (0.5 cycles/row)
with one-sided error compensation: the two DoubleRow k-tiles hold
(q_hi*k8 + q_lo*k8) = q*k8 where q_lo is the fp8 residual of q, so only
k's fp8 rounding error survives (iid across keys, it averages out in the
PV reduction; a q-side error would be correlated along its whole softmax
row and would not). PV runs in bf16:
  xt = x[b].T (host-transposed)      [1024, 2048]
  Q^T -> q8 fp8 [128, 2, N] (+bq; both k-tiles = q_hi)
  K^T -> k8 fp8 [128, 2, N] (t0 = k_hi, t1 = k - k_hi), V natural bf16
  S^T[k,q] = DoubleRow(k8_h, q8_h)   2 heads packed on partitions 0:64/64:128
  P = exp(S^T / 32)                  ScalarE, scale fused, bf16 out
  O'^T[d+1,q] = [V|ones].T @ P       ones column gives softmax denominators
  O^T = O'[0:64] * (1/denom)         recip_approx_fast + gpsimd broadcast
  out = O^T.T @ Wo_g                 [2048, 1024] partial, DMA'd out
Proj/wo chains are interleaved into the attention blocks as PE fillers so
the tensor engine never idles while ScalarE streams the exp() of the
4*2048*2048 attention matrix.
"""

import numpy as np

B = 2
N = 2048
E = 1024
HEADS = 16
D = 64
P = 128
NCORES = 8
GROUPS = 4            # TP groups
DG = E // GROUPS      # 256 cols per core
ECH = E // P          # 8 contraction chunks
NCH = N // P          # 16 sequence chunks
QS = 1024             # q span for softmax tiles
QB = 512              # matmul moving free dim

_CACHE = {}


def _round_f32r(x: np.ndarray) -> np.ndarray:
    """Round fp32 to fp32r (e8m11): RNE on the low 12 mantissa bits."""
    u = np.ascontiguousarray(x, dtype=np.float32).view(np.uint32)
    lower = u & np.uint32(0xFFF)
    base = u & np.uint32(0xFFFFF000)
    up = (lower > np.uint32(1 << 11)) | (
        (lower == np.uint32(1 << 11)) & (((base >> np.uint32(12)) & np.uint32(1)) == 1)
    )
    return (base + np.where(up, np.uint32(1 << 12), np.uint32(0))).view(np.float32)


def _build():
    import sys
    if "/opt/trn_rl_repo" not in sys.path:
        sys.path.insert(0, "/opt/trn_rl_repo")
    import concourse.tile as tile
    from concourse import bacc, mybir
    from concourse.bass import ts

    F32 = mybir.dt.float32
    F32R = mybir.dt.float32r
    BF16 = mybir.dt.bfloat16
    FP8 = mybir.dt.float8e4
    DR = mybir.MatmulPerfMode.DoubleRow
    Exp = mybir.ActivationFunctionType.Exp

    nc = bacc.Bacc("TRN2", target_bir_lowering=False, debug=False, num_devices=NCORES)

    xt = nc.dram_tensor("xt", [E, N], F32R, kind="ExternalInput").ap()
    wq = nc.dram_tensor("wq", [E, DG], F32R, kind="ExternalInput").ap()
    wk = nc.dram_tensor("wk", [E, DG], F32R, kind="ExternalInput").ap()
    wv = nc.dram_tensor("wv", [E, DG], F32R, kind="ExternalInput").ap()
    wo = nc.dram_tensor("wo", [DG, E], F32R, kind="ExternalInput").ap()
    bq2 = nc.dram_tensor("bq2", [P, 2], F32, kind="ExternalInput").ap()
    out = nc.dram_tensor("out", [N, E], F32, kind="ExternalOutput").ap()

    with tile.TileContext(nc) as tc:
        with tc.tile_pool(name="persist", bufs=1) as pers, \
             tc.tile_pool(name="pexp", bufs=12) as pexp_pool, \
             tc.tile_pool(name="small", bufs=2) as small, \
             tc.tile_pool(name="ostage", bufs=6) as ostage, \
             tc.tile_pool(name="ppmain", bufs=1, space="PSUM") as ppm, \
             tc.tile_pool(name="ppoacc", bufs=1, space="PSUM") as ppo:
            wq_sb = pers.tile([P, ECH, DG], F32R, tag="wq")
            wk_sb = pers.tile([P, ECH, DG], F32R, tag="wk")
            wv_sb = pers.tile([P, ECH, DG], F32R, tag="wv")
            wo_sb = pers.tile([P, 2, E], F32R, tag="wo")
            bq_sb = pers.tile([P, 2], F32, tag="bq")
            # fp8 DoubleRow tiles: [128 part = 2 heads x 64 d, 2 k-tiles, N]
            q8_p = [pers.tile([P, 2, N], FP8, tag=f"q8{i}", name=f"q8{i}") for i in range(2)]
            k8_p = [pers.tile([P, 2, N], FP8, tag=f"k8{i}", name=f"k8{i}") for i in range(2)]
            v_sb = pers.tile([P, NCH, GROUPS, 66], BF16, tag="v")
            oT_p = [pers.tile([P, N], F32R, tag=f"oT{i}", name=f"oT{i}") for i in range(2)]

            def proj_ps(i, name):
                return ppm.tile([P, QS], F32, tag="A" if i % 2 == 0 else "B", name=name)

            def k_chain(pair, qb, xt_sb):
                """K proj -> k8[pair]: both k-tiles = fp8(k). K's fp8 error is
                iid across keys so it averages out in PV; Q's would not."""
                def emit():
                    ps = proj_ps(qb, f"kps{pair}{qb}")
                    psl = ps[:, :QB]
                    for ec in range(ECH):
                        nc.tensor.matmul(
                            psl,
                            wk_sb[:, ec, ts(pair, P)],
                            xt_sb[:, ec, ts(qb, QB)],
                            start=(ec == 0), stop=(ec == ECH - 1),
                        )
                    nc.scalar.copy(
                        k8_p[pair][:, :, ts(qb, QB)],
                        psl[:, None, :].to_broadcast((P, 2, QB)),
                    )
                return emit

            def q_chain(pair, qb, xt_sb):
                """Q proj -> q8[pair]: t0 = fp8(q + bq), t1 = fp8 residual, so
                S = (q_hi + q_lo)*k = q*k up to k's fp8 rounding only."""
                def emit():
                    ps = proj_ps(qb, f"qps{pair}{qb}")
                    psl = ps[:, :QB]
                    for ec in range(ECH):
                        nc.tensor.matmul(
                            psl,
                            wq_sb[:, ec, ts(pair, P)],
                            xt_sb[:, ec, ts(qb, QB)],
                            start=(ec == 0), stop=(ec == ECH - 1),
                        )
                    dst = q8_p[pair]
                    nc.vector.tensor_add(
                        dst[:, 0, ts(qb, QB)],
                        psl,
                        bq_sb[:, pair, None].to_broadcast((P, QB)),
                    )
                    nc.vector.scalar_tensor_tensor(
                        dst[:, 1, ts(qb, QB)],
                        psl,
                        bq_sb[:, pair, None],
                        dst[:, 0, ts(qb, QB)],
                        mybir.AluOpType.add,
                        mybir.AluOpType.subtract,
                    )
                return emit

            def wo_chain(ncx, fb):
                def emit():
                    ps = proj_ps(ncx * 2 + fb, f"wops{ncx}{fb}")
                    psl = ps[:, :QB]
                    for dc in range(2):
                        nc.tensor.matmul(
                            psl,
                            oT_p[dc][:, ts(ncx, P)],
                            wo_sb[:, dc, ts(fb, QB)],
                            start=(dc == 0), stop=(dc == 1),
                        )
                    ot = ostage.tile([P, QB], F32, tag="ot", name="ot")
                    nc.vector.tensor_copy(ot, psl)
                    nc.sync.dma_start(out[ts(ncx, P), ts(fb, QB)], ot)
                return emit

            def v_chunk(ncx, xt_sb):
                """V proj chunk -> v_sb[:, ncx]. Uses ppm banks only (safe as
                an attention filler: ppo banks hold live PV accumulators)."""
                def emit():
                    ps = ppm.tile([P, QS], F32, tag="AB"[ncx % 2], name=f"vps{ncx}")
                    psl = ps[:, :DG]
                    for ec in range(ECH):
                        nc.tensor.matmul(
                            psl,
                            xt_sb[:, ec, ts(ncx, P)],
                            wv_sb[:, ec, :],
                            start=(ec == 0), stop=(ec == ECH - 1),
                        )
                    nc.scalar.copy(
                        v_sb[:, ncx, :, 0:64],
                        psl.rearrange("p (h d) -> p h d", d=D),
                    )
                return emit

            def emit_attn(pair, qs, fillers=(), fill_start=1, fill_every=1):
                fillers = list(fillers)
                oaccs = [ppo.tile([65, QS], F32, tag=f"O{h}", name=f"oacc{h}")
                         for h in range(2)]
                for kc in range(NCH):
                    if fillers and kc >= fill_start and (kc - fill_start) % fill_every == 0:
                        fillers.pop(0)()
                    pss = [ppm.tile([P, QS], F32, tag="AB"[h], name=f"spsum{h}")
                           for h in range(2)]
                    for h in range(2):
                        psl = slice(D * h, D * h + D)
                        for qb in range(QS // QB):
                            nc.tensor.matmul(
                                pss[h][:, ts(qb, QB)],
                                k8_p[pair][psl, :, ts(kc, P)],
                                q8_p[pair][psl, :, qs * QS + qb * QB:qs * QS + (qb + 1) * QB],
                                start=True, stop=True,
                                perf_mode=DR,
                            )
                    for h in range(2):
                        pe = pexp_pool.tile([P, QS], BF16, tag="pexp", name="pexp")
                        nc.scalar.activation(pe, pss[h], Exp, scale=1.0 / 32.0)
                        hh = 2 * pair + h
                        for qb in range(QS // QB):
                            nc.tensor.matmul(
                                oaccs[h][:, ts(qb, QB)],
                                v_sb[:, kc, hh, 0:65],
                                pe[:, ts(qb, QB)],
                                start=(kc == 0), stop=(kc == NCH - 1),
                            )
                for f in fillers:
                    f()
                # normalize: oT = O'[0:64] / denom, reading O' straight from PSUM
                d2 = small.tile([33, QS], F32, tag="d2", name="d2", bufs=1)
                for h in range(2):
                    nc.vector.tensor_copy(d2[32 * h:32 * h + 1, :], oaccs[h][64:65, :])
                r2 = small.tile([33, QS], F32, tag="r2", name="r2", bufs=1)
                nc.vector.reciprocal_approx_fast(r2, d2)
                rv1 = small.tile([1, QS], F32, tag="rv1", name="rv1", bufs=1)
                nc.vector.tensor_copy(rv1, r2[32:33, :])
                for h in range(2):
                    psl = slice(D * h, D * h + D)
                    rbc = small.tile([P, QS], F32, tag="rbc", name="rbc")
                    nc.gpsimd.partition_broadcast(rbc, r2[0:1, :] if h == 0 else rv1)
                    nc.vector.tensor_mul(
                        oT_p[pair][psl, ts(qs, QS)],
                        oaccs[h][0:64, :],
                        rbc[0:64, :],
                    )

            with tc.tile_pool(name="xtp", bufs=1) as xtp:
                xt_sb = xtp.tile([P, ECH, N], F32R, tag="xt")
                nc.sync.dma_start(wk_sb, wk.rearrange("(c p) d -> p c d", p=P))
                nc.sync.dma_start(wq_sb, wq.rearrange("(c p) d -> p c d", p=P))
                nc.sync.dma_start(wv_sb, wv.rearrange("(c p) d -> p c d", p=P))
                nc.sync.dma_start(wo_sb, wo.rearrange("(c p) f -> p c f", p=P))
                nc.sync.dma_start(bq_sb, bq2)
                xt_r = xt.rearrange("(c p) n -> p c n", p=P)
                for ncx in range(NCH):
                    nc.sync.dma_start(xt_sb[:, :, ts(ncx, P)], xt_r[:, :, ts(ncx, P)])
                ones_f32 = pers.tile([P, 1], F32, tag="ones")
                nc.vector.memset(ones_f32, 1.0)
                nc.vector.tensor_copy(
                    v_sb[:, :, :, 64:65],
                    ones_f32[:, 0, None, None, None].to_broadcast((P, NCH, GROUPS, 1)),
                )
                # upfront: K/Q pair 0 (S of block 1 needs all K, first-half Q),
                # V chunks 0..3 (PV kc 0..3); the rest stream in as fillers.
                for qb in range(4):
                    k_chain(0, qb, xt_sb)()
                for qb in range(2):
                    q_chain(0, qb, xt_sb)()
                for ncx in range(4):
                    v_chunk(ncx, xt_sb)()
                emit_attn(0, 0, fillers=(
                    [q_chain(0, qb, xt_sb) for qb in range(2, 4)]
                    + [v_chunk(ncx, xt_sb) for ncx in range(4, 16)]
                    + [k_chain(1, qb, xt_sb) for qb in range(2)]
                ))
                emit_attn(0, 1, fillers=(
                    [k_chain(1, qb, xt_sb) for qb in range(2, 4)]
                    + [q_chain(1, qb, xt_sb) for qb in range(4)]
                ), fill_every=2)
            emit_attn(1, 0)
            emit_attn(1, 1, fillers=(
                [wo_chain(ncx, fb) for ncx in range(8) for fb in range(2)]
            ))
            for ncx in range(8, NCH):
                for fb in range(2):
                    wo_chain(ncx, fb)()

    nc.compile()
    return nc


def _get_nc():
    if "nc" not in _CACHE:
        _CACHE["nc"] = _build()
    return _CACHE["nc"]


def kernel(x, Wq, bq, Wk, bk, Wv, bv, Wo, bo, **run_kwargs):
    import sys
    if "/opt/trn_rl_repo" not in sys.path:
        sys.path.insert(0, "/opt/trn_rl_repo")
    from concourse.bass_utils import run_bass_kernel_spmd

    x = np.asarray(x, dtype=np.float32)
    Wq = np.asarray(Wq, dtype=np.float32)
    Wk = np.asarray(Wk, dtype=np.float32)
    Wv = np.asarray(Wv, dtype=np.float32)
    Wo = np.asarray(Wo, dtype=np.float32)
    bq = np.asarray(bq, dtype=np.float32)
    bv = np.asarray(bv, dtype=np.float32)
    bo = np.asarray(bo, dtype=np.float32)

    nc = _get_nc()

    in_maps = []
    xts = [_round_f32r(np.ascontiguousarray(x[b].T)) for b in range(B)]
    for c in range(NCORES):
        b, g = divmod(c, GROUPS)
        cols = slice(g * DG, (g + 1) * DG)
        in_maps.append({
            "xt": xts[b],
            "wq": _round_f32r(Wq[:, cols]),
            "wk": _round_f32r(Wk[:, cols]),
            "wv": _round_f32r(Wv[:, cols]),
            "wo": _round_f32r(Wo[cols, :]),
            "bq2": np.ascontiguousarray(bq[cols].reshape(2, P).T),
        })

    try:
        res = run_bass_kernel_spmd(nc, in_maps, core_ids=list(range(NCORES)), **run_kwargs)
    except Exception:
        # device may be wedged from a prior run; reset the accelerator once
        try:
            import ctypes
            lib = ctypes.CDLL("/opt/axon/libaxon_pjrt.so")
            lib.axon_reset.restype = ctypes.c_int
            lib.axon_reset()
        except Exception:
            pass
        res = run_bass_kernel_spmd(nc, in_maps, core_ids=list(range(NCORES)), **run_kwargs)
    if run_kwargs:
        _CACHE["last_results"] = res

    # gather: sum TP partials per batch, add separable bias terms
    bias_vec = bv @ Wo + bo  # softmax rows sum to 1 => bv contributes bv@Wo
    full = np.empty((B, N, E), dtype=np.float32)
    for b in range(B):
        acc = res.results[b * GROUPS]["out"].astype(np.float32).copy()
        for g in range(1, GROUPS):
            acc += res.results[b * GROUPS + g]["out"]
        full[b] = acc + bias_vec[None, :]
    return full


# revision 5
# speedup vs baseline: 1.1510x; 1.1510x over previous
"""Multi-head attention Trainium2 kernel (8 NeuronCores).

Problem: x[2,2048,1024] -> MHA(16 heads, d=64) -> out[2,2048,1024], fp32.

Sharding: 2-way data parallel on batch x 4-way tensor parallel on heads.
Core c handles batch c//4 and heads 4*(c%4) .. 4*(c%4)+3 (a 256-wide slice
of the Wq/Wk/Wv columns and Wo rows). Each core returns a partial output
[2048,1024]; the host sums the 4 TP partials per batch and adds the bias
terms (bo, and bv@Wo which is separable because softmax rows sum to 1;
bk drops out of softmax entirely since (q+bq)@bk is constant along keys).

On-core dataflow (projections fp32r, attention bf16):
  xt = x[b].T (host-transposed)      [1024, 2048]
  Q^T = Wq_g^T stationary over xt    [256, 2048]  (+bq, d on partitions)
  K^T likewise (no bias), V natural  [2048, 256]  via xt-stationary matmuls
  S^T[k,q] = K^T(d,k).T @ Q^T(d,q)   2 heads row-packed (d=64 each)
  P = exp(S^T / 32)                  ScalarE, scale fused, bf16 out
  O'^T[d+1,q] = [V|ones].T @ P       ones column gives softmax denominators
  O^T = O'[0:64] * (1/denom)         recip_approx_fast + gpsimd broadcast
  out = O^T.T @ Wo_g                 [2048, 1024] partial, DMA'd out

Scheduling: the TensorE runs DVFS p-states — it only sustains its fast
rate (~0.42 ns/row) while continuously busy; any bubble drops it to a
~1.3-2.5x slower state. ScalarE's exp() of the 4*2048*2048 score matrix
(~550ns per [128,512] tile) is longer per kc step than the S+PV matmuls,
so a naive schedule bubbles the PE every step and equilibrates at the
slow clock. To stay dense:
  - attention runs in [QS=512]-wide q blocks; per (kc, head) S is a
    single 512-free matmul into one of 4 parity-rotated single-bank PSUM
    tiles, so S(kc) never waits on EXP(kc-1) bank reads;
  - PV is emitted lagged one kc behind S/EXP, so its dependence on
    EXP(kc) is already satisfied when the PE reaches it — no stall;
  - two PSUM banks are reserved for filler chains (QK/V projections, Wo
    output matmuls), which are metered into every attention block
    between the S group and the lagged PV group to absorb the leftover
    per-kc PE slack and keep the clock pinned at the fast state.
"""

import numpy as np

B = 2
N = 2048
E = 1024
HEADS = 16
D = 64
P = 128
NCORES = 8
GROUPS = 4            # TP groups
DG = E // GROUPS      # 256 cols per core
ECH = E // P          # 8 contraction chunks
NCH = N // P          # 16 sequence chunks
QS = 512              # q span per attention block == matmul free dim
NQB = N // QS         # 4 q blocks per pair

_CACHE = {}


def _round_f32r(x: np.ndarray) -> np.ndarray:
    """Round fp32 to fp32r (e8m11): RNE on the low 12 mantissa bits."""
    u = np.ascontiguousarray(x, dtype=np.float32).view(np.uint32)
    lower = u & np.uint32(0xFFF)
    base = u & np.uint32(0xFFFFF000)
    up = (lower > np.uint32(1 << 11)) | (
        (lower == np.uint32(1 << 11)) & (((base >> np.uint32(12)) & np.uint32(1)) == 1)
    )
    return (base + np.where(up, np.uint32(1 << 12), np.uint32(0))).view(np.float32)


def _build():
    import sys
    if "/opt/trn_rl_repo" not in sys.path:
        sys.path.insert(0, "/opt/trn_rl_repo")
    import concourse.tile as tile
    from concourse import bacc, mybir
    from concourse.bass import ts

    F32 = mybir.dt.float32
    F32R = mybir.dt.float32r
    BF16 = mybir.dt.bfloat16
    Exp = mybir.ActivationFunctionType.Exp

    nc = bacc.Bacc("TRN2", target_bir_lowering=False, debug=False, num_devices=NCORES)

    xt = nc.dram_tensor("xt", [E, N], F32R, kind="ExternalInput").ap()
    wq = nc.dram_tensor("wq", [E, DG], F32R, kind="ExternalInput").ap()
    wk = nc.dram_tensor("wk", [E, DG], F32R, kind="ExternalInput").ap()
    wv = nc.dram_tensor("wv", [E, DG], F32R, kind="ExternalInput").ap()
    wo = nc.dram_tensor("wo", [DG, E], F32R, kind="ExternalInput").ap()
    bq2 = nc.dram_tensor("bq2", [P, 2], F32, kind="ExternalInput").ap()
    out = nc.dram_tensor("out", [N, E], F32, kind="ExternalOutput").ap()

    with tile.TileContext(nc) as tc:
        with tc.tile_pool(name="persist", bufs=1) as pers, \
             tc.tile_pool(name="pexp", bufs=12) as pexp_pool, \
             tc.tile_pool(name="small", bufs=2) as small, \
             tc.tile_pool(name="ostage", bufs=6) as ostage, \
             tc.tile_pool(name="ppmain", bufs=1, space="PSUM") as ppm, \
             tc.tile_pool(name="ppfill", bufs=1, space="PSUM") as ppf, \
             tc.tile_pool(name="ppoacc", bufs=1, space="PSUM") as ppo:
            wq_sb = pers.tile([P, ECH, DG], F32R, tag="wq")
            wk_sb = pers.tile([P, ECH, DG], F32R, tag="wk")
            wv_sb = pers.tile([P, ECH, DG], F32R, tag="wv")
            wo_sb = pers.tile([P, 2, E], F32R, tag="wo")
            bq_sb = pers.tile([P, 2], F32, tag="bq")
            qT_p = [pers.tile([P, N], BF16, tag=f"qT{i}", name=f"qT{i}") for i in range(2)]
            kT_p = [pers.tile([P, N], BF16, tag=f"kT{i}", name=f"kT{i}") for i in range(2)]
            v_sb = pers.tile([P, NCH, GROUPS, 66], BF16, tag="v")
            oT_p = [pers.tile([P, N], F32R, tag=f"oT{i}", name=f"oT{i}") for i in range(2)]

            def fill_ps(i, name):
                return ppf.tile([P, QS], F32, tag="C" if i % 2 == 0 else "D", name=name)

            def k_chain(pair, qb):
                def emit():
                    ps = fill_ps(qb, f"kps{pair}{qb}")
                    for ec in range(ECH):
                        nc.tensor.matmul(
                            ps,
                            wk_sb[:, ec, ts(pair, P)],
                            _xt()[:, ec, ts(qb, QS)],
                            start=(ec == 0), stop=(ec == ECH - 1),
                        )
                    nc.scalar.copy(kT_p[pair][:, ts(qb, QS)], ps)
                return emit

            def q_chain(pair, qb):
                def emit():
                    ps = fill_ps(qb, f"qps{pair}{qb}")
                    for ec in range(ECH):
                        nc.tensor.matmul(
                            ps,
                            wq_sb[:, ec, ts(pair, P)],
                            _xt()[:, ec, ts(qb, QS)],
                            start=(ec == 0), stop=(ec == ECH - 1),
                        )
                    nc.vector.tensor_add(
                        qT_p[pair][:, ts(qb, QS)], ps,
                        bq_sb[:, pair, None].to_broadcast((P, QS)),
                    )
                return emit

            def v_chunk(ncx):
                def emit():
                    ps = fill_ps(ncx, f"vps{ncx}")
                    psl = ps[:, :DG]
                    for ec in range(ECH):
                        nc.tensor.matmul(
                            psl,
                            _xt()[:, ec, ts(ncx, P)],
                            wv_sb[:, ec, :],
                            start=(ec == 0), stop=(ec == ECH - 1),
                        )
                    nc.scalar.copy(
                        v_sb[:, ncx, :, 0:64],
                        psl.rearrange("p (h d) -> p h d", d=D),
                    )
                return emit

            def wo_chain(ncx, fb):
                def emit():
                    ps = fill_ps(ncx * 2 + fb, f"wops{ncx}{fb}")
                    for dc in range(2):
                        nc.tensor.matmul(
                            ps,
                            oT_p[dc][:, ts(ncx, P)],
                            wo_sb[:, dc, ts(fb, QS)],
                            start=(dc == 0), stop=(dc == 1),
                        )
                    ot = ostage.tile([P, QS], F32, tag="ot", name="ot")
                    nc.vector.tensor_copy(ot, ps)
                    nc.sync.dma_start(out[ts(ncx, P), ts(fb, QS)], ot)
                return emit

            def emit_attn(pair, qs, fillers=()):
                """One [QS]-wide q block: S/EXP stream with PV lagged one kc;
                fillers run on their own PSUM banks between S and lagged PV."""
                fillers = list(fillers)
                step = max(1, NCH // (len(fillers) + 1)) if fillers else NCH + 1
                oaccs = [ppo.tile([65, QS], F32, tag=f"O{h}", name=f"oacc{h}")
                         for h in range(2)]
                prev_pes = None
                for kc in range(NCH + 1):
                    pes = None
                    if kc < NCH:
                        pes = []
                        for h in range(2):
                            psl = slice(D * h, D * h + D)
                            ps = ppm.tile([P, QS], F32, tag=f"S{h}{kc % 2}",
                                          name=f"spsum{h}")
                            nc.tensor.matmul(
                                ps,
                                kT_p[pair][psl, ts(kc, P)],
                                qT_p[pair][psl, ts(qs, QS)],
                                start=True, stop=True,
                            )
                            pe = pexp_pool.tile([P, QS], BF16, tag="pexp", name="pexp")
                            nc.scalar.activation(pe, ps, Exp, scale=1.0 / 32.0)
                            pes.append(pe)
                        if fillers and kc % step == 0:
                            fillers.pop(0)()
                    if prev_pes is not None:
                        kp = kc - 1
                        for h in range(2):
                            nc.tensor.matmul(
                                oaccs[h][:, :],
                                v_sb[:, kp, 2 * pair + h, 0:65],
                                prev_pes[h],
                                start=(kp == 0), stop=(kp == NCH - 1),
                            )
                    prev_pes = pes
                for f in fillers:
                    f()
                # normalize: oT = O'[0:64] / denom, reading O' straight from PSUM
                d2 = small.tile([33, QS], F32, tag="d2", name="d2", bufs=1)
                for h in range(2):
                    nc.vector.tensor_copy(d2[32 * h:32 * h + 1, :], oaccs[h][64:65, :])
                r2 = small.tile([33, QS], F32, tag="r2", name="r2", bufs=1)
                nc.vector.reciprocal_approx_fast(r2, d2)
                rv1 = small.tile([1, QS], F32, tag="rv1", name="rv1", bufs=1)
                nc.vector.tensor_copy(rv1, r2[32:33, :])
                for h in range(2):
                    psl = slice(D * h, D * h + D)
                    rbc = small.tile([D, QS], F32, tag="rbc", name="rbc")
                    nc.gpsimd.partition_broadcast(rbc, r2[0:1, :] if h == 0 else rv1)
                    nc.vector.tensor_mul(
                        oT_p[pair][psl, ts(qs, QS)],
                        oaccs[h][0:64, :],
                        rbc,
                    )

            with tc.tile_pool(name="xtp", bufs=1) as xtp:
                xt_sb = xtp.tile([P, ECH, N], F32R, tag="xt")
                _xt = lambda: xt_sb
                nc.sync.dma_start(wk_sb, wk.rearrange("(c p) d -> p c d", p=P))
                nc.sync.dma_start(wq_sb, wq.rearrange("(c p) d -> p c d", p=P))
                nc.sync.dma_start(wv_sb, wv.rearrange("(c p) d -> p c d", p=P))
                nc.sync.dma_start(wo_sb, wo.rearrange("(c p) f -> p c f", p=P))
                nc.sync.dma_start(bq_sb, bq2)
                xt_r = xt.rearrange("(c p) n -> p c n", p=P)
                for ncx in range(NCH):
                    nc.sync.dma_start(xt_sb[:, :, ts(ncx, P)], xt_r[:, :, ts(ncx, P)])
                ones_f32 = pers.tile([P, 1], F32, tag="ones")
                nc.vector.memset(ones_f32, 1.0)
                nc.vector.tensor_copy(
                    v_sb[:, :, :, 64:65],
                    ones_f32[:, 0, None, None, None].to_broadcast((P, NCH, GROUPS, 1)),
                )
                # upfront: all K pair 0, first Q block, V chunks 0..3
                for qb in range(NQB):
                    k_chain(0, qb)()
                q_chain(0, 0)()
                for ncx in range(4):
                    v_chunk(ncx)()
                emit_attn(0, 0, fillers=(
                    [v_chunk(ncx) for ncx in range(4, NCH)] + [q_chain(0, 1)]
                ))
                emit_attn(0, 1, fillers=[q_chain(0, 2), k_chain(1, 0), k_chain(1, 1)])
                emit_attn(0, 2, fillers=[q_chain(0, 3), k_chain(1, 2), k_chain(1, 3)])
                emit_attn(0, 3, fillers=[q_chain(1, 0), q_chain(1, 1)])
                emit_attn(1, 0, fillers=[q_chain(1, 2), q_chain(1, 3)])
            emit_attn(1, 1, fillers=[wo_chain(ncx, fb) for ncx in range(0, 4)
                                     for fb in range(2)])
            emit_attn(1, 2, fillers=[wo_chain(ncx, fb) for ncx in range(4, 8)
                                     for fb in range(2)])
            emit_attn(1, 3, fillers=[wo_chain(ncx, fb) for ncx in range(8, 12)
                                     for fb in range(2)])
            for ncx in range(12, NCH):
                for fb in range(2):
                    wo_chain(ncx, fb)()

    nc.compile()
    return nc


def _get_nc():
    if "nc" not in _CACHE:
        _CACHE["nc"] = _build()
    return _CACHE["nc"]


def kernel(x, Wq, bq, Wk, bk, Wv, bv, Wo, bo, **run_kwargs):
    import sys
    if "/opt/trn_rl_repo" not in sys.path:
        sys.path.insert(0, "/opt/trn_rl_repo")
    from concourse.bass_utils import run_bass_kernel_spmd

    x = np.asarray(x, dtype=np.float32)
    Wq = np.asarray(Wq, dtype=np.float32)
    Wk = np.asarray(Wk, dtype=np.float32)
    Wv = np.asarray(Wv, dtype=np.float32)
    Wo = np.asarray(Wo, dtype=np.float32)
    bq = np.asarray(bq, dtype=np.float32)
    bv = np.asarray(bv, dtype=np.float32)
    bo = np.asarray(bo, dtype=np.float32)

    nc = _get_nc()

    in_maps = []
    xts = [_round_f32r(np.ascontiguousarray(x[b].T)) for b in range(B)]
    for c in range(NCORES):
        b, g = divmod(c, GROUPS)
        cols = slice(g * DG, (g + 1) * DG)
        in_maps.append({
            "xt": xts[b],
            "wq": _round_f32r(Wq[:, cols]),
            "wk": _round_f32r(Wk[:, cols]),
            "wv": _round_f32r(Wv[:, cols]),
            "wo": _round_f32r(Wo[cols, :]),
            "bq2": np.ascontiguousarray(bq[cols].reshape(2, P).T),
        })

    try:
        res = run_bass_kernel_spmd(nc, in_maps, core_ids=list(range(NCORES)), **run_kwargs)
    except Exception:
        # device may be wedged from a prior run; reset the accelerator once
        try:
            import ctypes
            lib = ctypes.CDLL("/opt/axon/libaxon_pjrt.so")
            lib.axon_reset.restype = ctypes.c_int
            lib.axon_reset()
        except Exception:
            pass
        res = run_bass_kernel_spmd(nc, in_maps, core_ids=list(range(NCORES)), **run_kwargs)
    if run_kwargs:
        _CACHE["last_results"] = res

    # gather: sum TP partials per batch, add separable bias terms
    bias_vec = bv @ Wo + bo  # softmax rows sum to 1 => bv contributes bv@Wo
    full = np.empty((B, N, E), dtype=np.float32)
    for b in range(B):
        acc = res.results[b * GROUPS]["out"].astype(np.float32).copy()
        for g in range(1, GROUPS):
            acc += res.results[b * GROUPS + g]["out"]
        full[b] = acc + bias_vec[None, :]
    return full


# revision 10
# speedup vs baseline: 1.3112x; 1.1392x over previous
"""Multi-head attention Trainium2 kernel (8 NeuronCores).

Problem: x[2,2048,1024] -> MHA(16 heads, d=64) -> out[2,2048,1024], fp32.

Sharding: 2-way data parallel on batch x 4-way tensor parallel on heads.
Core c handles batch c//4 and heads 4*(c%4) .. 4*(c%4)+3 (a 256-wide slice
of the Wq/Wk/Wv columns and Wo rows). Each core returns a partial output
[2048,1024]; the host sums the 4 TP partials per batch and adds the bias
terms (bo, and bv@Wo which is separable because softmax rows sum to 1;
bk drops out of softmax entirely since (q+bq)@bk is constant along keys).

On-core dataflow (projections fp32r, attention bf16):
  xt = x[b].T (host-transposed)      [1024, 2048]
  Q^T = Wq_g^T stationary over xt    [256, 2048]  (+bq, d on partitions)
  K^T likewise (no bias), V natural  [2048, 256]  via xt-stationary matmuls
  S^T[k,q] = K^T(d,k).T @ Q^T(d,q)   2 heads row-packed (d=64 each)
  P = exp(S^T / 32)                  ScalarE, scale fused, bf16 out
  O'^T[d+1,q] = [V|ones].T @ P       ones column gives softmax denominators
  O^T = O'[0:64] * (1/denom)         recip_approx_fast + gpsimd broadcast
  out = O^T.T @ Wo_g                 [2048, 1024] partial, DMA'd out

Scheduling: the TensorE runs DVFS p-states — it only sustains its fast
rate (~0.42 ns/row) while continuously busy; any bubble drops it to a
~1.3-2.5x slower state. ScalarE's exp() of the 4*2048*2048 score matrix
(~550ns per [128,512] tile) is longer per kc step than the S+PV matmuls,
so a naive schedule bubbles the PE every step and equilibrates at the
slow clock. To stay dense:
  - attention runs in [QS=512]-wide q blocks; per (kc, head) S is a
    single 512-free matmul into one of 4 parity-rotated single-bank PSUM
    tiles, so S(kc) never waits on EXP(kc-1) bank reads;
  - PV is emitted lagged one kc behind S/EXP, so its dependence on
    EXP(kc) is already satisfied when the PE reaches it — no stall;
  - two PSUM banks are reserved for filler chains (QK/V projections, Wo
    output matmuls), which are metered into every attention block
    between the S group and the lagged PV group to absorb the leftover
    per-kc PE slack and keep the clock pinned at the fast state.
"""

import numpy as np

B = 2
N = 2048
E = 1024
HEADS = 16
D = 64
P = 128
NCORES = 8
GROUPS = 4            # TP groups
DG = E // GROUPS      # 256 cols per core
ECH = E // P          # 8 contraction chunks
NCH = N // P          # 16 sequence chunks
QS = 512              # q span per attention block == matmul free dim
NQB = N // QS         # 4 q blocks per pair

_CACHE = {}


def _round_f32r(x: np.ndarray) -> np.ndarray:
    """Round fp32 to fp32r (e8m11): RNE on the low 12 mantissa bits."""
    u = np.ascontiguousarray(x, dtype=np.float32).view(np.uint32)
    lower = u & np.uint32(0xFFF)
    base = u & np.uint32(0xFFFFF000)
    up = (lower > np.uint32(1 << 11)) | (
        (lower == np.uint32(1 << 11)) & (((base >> np.uint32(12)) & np.uint32(1)) == 1)
    )
    return (base + np.where(up, np.uint32(1 << 12), np.uint32(0))).view(np.float32)


def _build():
    import sys
    if "/opt/trn_rl_repo" not in sys.path:
        sys.path.insert(0, "/opt/trn_rl_repo")
    import concourse.tile as tile
    from concourse import bacc, mybir
    from concourse.bass import ts

    F32 = mybir.dt.float32
    F32R = mybir.dt.float32r
    BF16 = mybir.dt.bfloat16
    Exp = mybir.ActivationFunctionType.Exp

    nc = bacc.Bacc("TRN2", target_bir_lowering=False, debug=False, num_devices=NCORES)

    xt = nc.dram_tensor("xt", [E, N], F32R, kind="ExternalInput").ap()
    wq = nc.dram_tensor("wq", [E, DG], F32R, kind="ExternalInput").ap()
    wk = nc.dram_tensor("wk", [E, DG], F32R, kind="ExternalInput").ap()
    wv = nc.dram_tensor("wv", [E, DG], F32R, kind="ExternalInput").ap()
    wo = nc.dram_tensor("wo", [DG, E], F32R, kind="ExternalInput").ap()
    bq2 = nc.dram_tensor("bq2", [P, 2], F32, kind="ExternalInput").ap()
    out = nc.dram_tensor("out", [N, E], F32, kind="ExternalOutput").ap()

    with tile.TileContext(nc) as tc:
        with tc.tile_pool(name="persist", bufs=1) as pers, \
             tc.tile_pool(name="pexp", bufs=12) as pexp_pool, \
             tc.tile_pool(name="small", bufs=2) as small, \
             tc.tile_pool(name="ostage", bufs=6) as ostage, \
             tc.tile_pool(name="ppmain", bufs=1, space="PSUM") as ppm, \
             tc.tile_pool(name="ppfill", bufs=1, space="PSUM") as ppf, \
             tc.tile_pool(name="ppoacc", bufs=1, space="PSUM") as ppo:
            wq_sb = pers.tile([P, ECH, DG], F32R, tag="wq")
            wk_sb = pers.tile([P, ECH, DG], F32R, tag="wk")
            wv_sb = pers.tile([P, ECH, DG], F32R, tag="wv")
            wo_sb = pers.tile([P, 2, E], F32R, tag="wo")
            bq_sb = pers.tile([P, 2], F32, tag="bq")
            qT_p = [pers.tile([P, N], BF16, tag=f"qT{i}", name=f"qT{i}") for i in range(2)]
            # kTz[pair][h]: K^T of head h in its 64-row half, zeros in the
            # other half. S then uses a full 128-row stationary (the zero rows
            # annihilate the other head in the shared 128-row qT moving data);
            # partial-row stationaries defeat the PE's weight-load double
            # buffering and cost ~100ns exposed LDWEIGHTS per matmul.
            kTz_p = [[pers.tile([P, N], BF16, tag=f"kTz{i}{h}", name=f"kTz{i}{h}")
                      for h in range(2)] for i in range(2)]
            v_sb = pers.tile([P, NCH, GROUPS, 66], BF16, tag="v")
            oT_p = [pers.tile([P, N], F32R, tag=f"oT{i}", name=f"oT{i}") for i in range(2)]

            def fill_ps(i, name):
                return ppf.tile([P, QS], F32, tag="C" if i % 2 == 0 else "D", name=name)

            def k_chain(pair, qb):
                def emit():
                    ps = fill_ps(qb, f"kps{pair}{qb}")
                    for ec in range(ECH):
                        nc.tensor.matmul(
                            ps,
                            wk_sb[:, ec, ts(pair, P)],
                            _xt()[:, ec, ts(qb, QS)],
                            start=(ec == 0), stop=(ec == ECH - 1),
                        )
                    for h in range(2):
                        psl = slice(D * h, D * h + D)
                        nc.vector.tensor_copy(
                            kTz_p[pair][h][psl, ts(qb, QS)], ps[psl, :])
                return emit

            def q_chain(pair, qb):
                def emit():
                    ps = fill_ps(qb, f"qps{pair}{qb}")
                    for ec in range(ECH):
                        nc.tensor.matmul(
                            ps,
                            wq_sb[:, ec, ts(pair, P)],
                            _xt()[:, ec, ts(qb, QS)],
                            start=(ec == 0), stop=(ec == ECH - 1),
                        )
                    nc.vector.tensor_add(
                        qT_p[pair][:, ts(qb, QS)], ps,
                        bq_sb[:, pair, None].to_broadcast((P, QS)),
                    )
                return emit

            def v_chunk(ncx):
                def emit():
                    ps = fill_ps(ncx, f"vps{ncx}")
                    psl = ps[:, :DG]
                    for ec in range(ECH):
                        nc.tensor.matmul(
                            psl,
                            _xt()[:, ec, ts(ncx, P)],
                            wv_sb[:, ec, :],
                            start=(ec == 0), stop=(ec == ECH - 1),
                        )
                    nc.vector.tensor_copy(
                        v_sb[:, ncx, :, 0:64],
                        psl.rearrange("p (h d) -> p h d", d=D),
                    )
                return emit

            def wo_chain(ncx, fb):
                def emit():
                    ps = fill_ps(ncx * 2 + fb, f"wops{ncx}{fb}")
                    for dc in range(2):
                        nc.tensor.matmul(
                            ps,
                            oT_p[dc][:, ts(ncx, P)],
                            wo_sb[:, dc, ts(fb, QS)],
                            start=(dc == 0), stop=(dc == 1),
                        )
                    ot = ostage.tile([P, QS], F32, tag="ot", name="ot")
                    nc.vector.tensor_copy(ot, ps)
                    nc.sync.dma_start(out[ts(ncx, P), ts(fb, QS)], ot)
                return emit

            def emit_attn(pair, qs, fillers=()):
                """One [QS]-wide q block: S/EXP stream with PV lagged one kc;
                fillers run on their own PSUM banks between S and lagged PV."""
                fillers = list(fillers)
                step = max(1, NCH // (len(fillers) + 1)) if fillers else NCH + 1
                oaccs = [ppo.tile([65, QS], F32, tag=f"O{h}", name=f"oacc{h}")
                         for h in range(2)]
                prev_pes = None
                for kc in range(NCH + 1):
                    pes = None
                    if kc < NCH:
                        pes = []
                        for h in range(2):
                            ps = ppm.tile([P, QS], F32, tag=f"S{h}{kc % 2}",
                                          name=f"spsum{h}")
                            nc.tensor.matmul(
                                ps,
                                kTz_p[pair][h][:, ts(kc, P)],
                                qT_p[pair][:, ts(qs, QS)],
                                start=True, stop=True,
                            )
                            pe = pexp_pool.tile([P, QS], BF16, tag="pexp", name="pexp")
                            nc.scalar.activation(pe, ps, Exp, scale=1.0 / 32.0)
                            pes.append(pe)
                        if fillers and kc % step == 0:
                            fillers.pop(0)()
                    if prev_pes is not None:
                        kp = kc - 1
                        for h in range(2):
                            nc.tensor.matmul(
                                oaccs[h][:, :],
                                v_sb[:, kp, 2 * pair + h, 0:65],
                                prev_pes[h],
                                start=(kp == 0), stop=(kp == NCH - 1),
                            )
                    prev_pes = pes
                for f in fillers:
                    f()
                # normalize: oT = O'[0:64] / denom, reading O' straight from PSUM
                d2 = small.tile([33, QS], F32, tag="d2", name="d2", bufs=1)
                for h in range(2):
                    nc.vector.tensor_copy(d2[32 * h:32 * h + 1, :], oaccs[h][64:65, :])
                r2 = small.tile([33, QS], F32, tag="r2", name="r2", bufs=1)
                nc.vector.reciprocal_approx_fast(r2, d2)
                rv1 = small.tile([1, QS], F32, tag="rv1", name="rv1", bufs=1)
                nc.vector.tensor_copy(rv1, r2[32:33, :])
                for h in range(2):
                    psl = slice(D * h, D * h + D)
                    rbc = small.tile([D, QS], F32, tag="rbc", name="rbc")
                    nc.gpsimd.partition_broadcast(rbc, r2[0:1, :] if h == 0 else rv1)
                    nc.vector.tensor_mul(
                        oT_p[pair][psl, ts(qs, QS)],
                        oaccs[h][0:64, :],
                        rbc,
                    )

            with tc.tile_pool(name="xtp", bufs=1) as xtp:
                xt_sb = xtp.tile([P, ECH, N], F32R, tag="xt")
                _xt = lambda: xt_sb
                nc.sync.dma_start(wk_sb, wk.rearrange("(c p) d -> p c d", p=P))
                nc.sync.dma_start(wq_sb, wq.rearrange("(c p) d -> p c d", p=P))
                nc.sync.dma_start(wv_sb, wv.rearrange("(c p) d -> p c d", p=P))
                nc.sync.dma_start(wo_sb, wo.rearrange("(c p) f -> p c f", p=P))
                nc.sync.dma_start(bq_sb, bq2)
                xt_r = xt.rearrange("(c p) n -> p c n", p=P)
                for ncx in range(NCH):
                    nc.sync.dma_start(xt_sb[:, :, ts(ncx, P)], xt_r[:, :, ts(ncx, P)])
                for pair in range(2):
                    nc.gpsimd.memset(kTz_p[pair][0][D:, :], 0.0)
                    nc.gpsimd.memset(kTz_p[pair][1][0:D, :], 0.0)
                ones_f32 = pers.tile([P, 1], F32, tag="ones")
                nc.vector.memset(ones_f32, 1.0)
                nc.vector.tensor_copy(
                    v_sb[:, :, :, 64:65],
                    ones_f32[:, 0, None, None, None].to_broadcast((P, NCH, GROUPS, 1)),
                )
                # upfront: all K pair 0, first Q block, V chunks 0..3
                for qb in range(NQB):
                    k_chain(0, qb)()
                q_chain(0, 0)()
                for ncx in range(4):
                    v_chunk(ncx)()
                emit_attn(0, 0, fillers=(
                    [v_chunk(ncx) for ncx in range(4, NCH)] + [q_chain(0, 1)]
                ))
                emit_attn(0, 1, fillers=[q_chain(0, 2), k_chain(1, 0), k_chain(1, 1)])
                emit_attn(0, 2, fillers=[q_chain(0, 3), k_chain(1, 2), k_chain(1, 3)])
                emit_attn(0, 3, fillers=[q_chain(1, 0), q_chain(1, 1)])
                emit_attn(1, 0, fillers=[q_chain(1, 2), q_chain(1, 3)])
            emit_attn(1, 1, fillers=[wo_chain(ncx, fb) for ncx in range(0, 4)
                                     for fb in range(2)])
            emit_attn(1, 2, fillers=[wo_chain(ncx, fb) for ncx in range(4, 8)
                                     for fb in range(2)])
            emit_attn(1, 3, fillers=[wo_chain(ncx, fb) for ncx in range(8, 12)
                                     for fb in range(2)])
            for ncx in range(12, NCH):
                for fb in range(2):
                    wo_chain(ncx, fb)()

    nc.compile()
    return nc


def _get_nc():
    if "nc" not in _CACHE:
        _CACHE["nc"] = _build()
    return _CACHE["nc"]


def kernel(x, Wq, bq, Wk, bk, Wv, bv, Wo, bo, **run_kwargs):
    import sys
    if "/opt/trn_rl_repo" not in sys.path:
        sys.path.insert(0, "/opt/trn_rl_repo")
    from concourse.bass_utils import run_bass_kernel_spmd

    x = np.asarray(x, dtype=np.float32)
    Wq = np.asarray(Wq, dtype=np.float32)
    Wk = np.asarray(Wk, dtype=np.float32)
    Wv = np.asarray(Wv, dtype=np.float32)
    Wo = np.asarray(Wo, dtype=np.float32)
    bq = np.asarray(bq, dtype=np.float32)
    bv = np.asarray(bv, dtype=np.float32)
    bo = np.asarray(bo, dtype=np.float32)

    nc = _get_nc()

    in_maps = []
    xts = [_round_f32r(np.ascontiguousarray(x[b].T)) for b in range(B)]
    for c in range(NCORES):
        b, g = divmod(c, GROUPS)
        cols = slice(g * DG, (g + 1) * DG)
        in_maps.append({
            "xt": xts[b],
            "wq": _round_f32r(Wq[:, cols]),
            "wk": _round_f32r(Wk[:, cols]),
            "wv": _round_f32r(Wv[:, cols]),
            "wo": _round_f32r(Wo[cols, :]),
            "bq2": np.ascontiguousarray(bq[cols].reshape(2, P).T),
        })

    try:
        res = run_bass_kernel_spmd(nc, in_maps, core_ids=list(range(NCORES)), **run_kwargs)
    except Exception:
        # device may be wedged from a prior run; reset the accelerator once
        try:
            import ctypes
            lib = ctypes.CDLL("/opt/axon/libaxon_pjrt.so")
            lib.axon_reset.restype = ctypes.c_int
            lib.axon_reset()
        except Exception:
            pass
        res = run_bass_kernel_spmd(nc, in_maps, core_ids=list(range(NCORES)), **run_kwargs)
    if run_kwargs:
        _CACHE["last_results"] = res

    # gather: sum TP partials per batch, add separable bias terms
    bias_vec = bv @ Wo + bo  # softmax rows sum to 1 => bv contributes bv@Wo
    full = np.empty((B, N, E), dtype=np.float32)
    for b in range(B):
        acc = res.results[b * GROUPS]["out"].astype(np.float32).copy()
        for g in range(1, GROUPS):
            acc += res.results[b * GROUPS + g]["out"]
        full[b] = acc + bias_vec[None, :]
    return full


# revision 19
# speedup vs baseline: 1.3276x; 1.0125x over previous
"""Multi-head attention Trainium2 kernel (8 NeuronCores).

Problem: x[2,2048,1024] -> MHA(16 heads, d=64) -> out[2,2048,1024], fp32.

Sharding: 2-way data parallel on batch x 4-way tensor parallel on heads.
Core c handles batch c//4 and heads 4*(c%4) .. 4*(c%4)+3 (a 256-wide slice
of the Wq/Wk/Wv columns and Wo rows). Each core returns a partial output
[2048,1024]; the host sums the 4 TP partials per batch and adds the bias
terms (bo, and bv@Wo which is separable because softmax rows sum to 1;
bk drops out of softmax entirely since (q+bq)@bk is constant along keys).

On-core dataflow (projections fp32r, attention bf16):
  xt = x[b].T (host-transposed)      [1024, 2048]
  Q^T = Wq_g^T stationary over xt    [256, 2048]  (+bq, d on partitions)
  K^T likewise (no bias), V natural  [2048, 256]  via xt-stationary matmuls
  S^T[k,q] = K^T(d,k).T @ Q^T(d,q)   2 heads row-packed (d=64 each)
  P = exp(S^T / 32)                  ScalarE, scale fused, bf16 out
  O'^T[d+1,q] = [V|ones].T @ P       ones column gives softmax denominators
  O^T = O'[0:64] * (1/denom)         recip_approx_fast + gpsimd broadcast
  out = O^T.T @ Wo_g                 [2048, 1024] partial, DMA'd out

Scheduling: the TensorE runs DVFS p-states — it only sustains its fast
rate (~0.42 ns/row) while continuously busy; any bubble drops it to a
~1.3-2.5x slower state. ScalarE's exp() of the 4*2048*2048 score matrix
(~550ns per [128,512] tile) is longer per kc step than the S+PV matmuls,
so a naive schedule bubbles the PE every step and equilibrates at the
slow clock. To stay dense:
  - attention runs in [QS=512]-wide q blocks; per (kc, head) S is a
    single 512-free matmul into one of 4 parity-rotated single-bank PSUM
    tiles, so S(kc) never waits on EXP(kc-1) bank reads;
  - PV is emitted lagged one kc behind S/EXP, so its dependence on
    EXP(kc) is already satisfied when the PE reaches it — no stall;
  - two PSUM banks are reserved for filler chains (QK/V projections, Wo
    output matmuls), which are metered into every attention block
    between the S group and the lagged PV group to absorb the leftover
    per-kc PE slack and keep the clock pinned at the fast state.
"""

import numpy as np

B = 2
N = 2048
E = 1024
HEADS = 16
D = 64
P = 128
NCORES = 8
GROUPS = 4            # TP groups
DG = E // GROUPS      # 256 cols per core
ECH = E // P          # 8 contraction chunks
NCH = N // P          # 16 sequence chunks
QS = 512              # q span per attention block == matmul free dim
NQB = N // QS         # 4 q blocks per pair

_CACHE = {}


def _round_f32r(x: np.ndarray) -> np.ndarray:
    """Round fp32 to fp32r (e8m11): RNE on the low 12 mantissa bits."""
    u = np.ascontiguousarray(x, dtype=np.float32).view(np.uint32)
    lower = u & np.uint32(0xFFF)
    base = u & np.uint32(0xFFFFF000)
    up = (lower > np.uint32(1 << 11)) | (
        (lower == np.uint32(1 << 11)) & (((base >> np.uint32(12)) & np.uint32(1)) == 1)
    )
    return (base + np.where(up, np.uint32(1 << 12), np.uint32(0))).view(np.float32)


def _build():
    import sys
    if "/opt/trn_rl_repo" not in sys.path:
        sys.path.insert(0, "/opt/trn_rl_repo")
    import concourse.tile as tile
    from concourse import bacc, mybir
    from concourse.bass import ts

    F32 = mybir.dt.float32
    F32R = mybir.dt.float32r
    BF16 = mybir.dt.bfloat16
    Exp = mybir.ActivationFunctionType.Exp

    nc = bacc.Bacc("TRN2", target_bir_lowering=False, debug=False, num_devices=NCORES)

    xt = nc.dram_tensor("xt", [E, N], F32R, kind="ExternalInput").ap()
    wq = nc.dram_tensor("wq", [E, DG], F32R, kind="ExternalInput").ap()
    wk = nc.dram_tensor("wk", [E, DG], F32R, kind="ExternalInput").ap()
    wv = nc.dram_tensor("wv", [E, DG], F32R, kind="ExternalInput").ap()
    wo = nc.dram_tensor("wo", [DG, E], F32R, kind="ExternalInput").ap()
    bq2 = nc.dram_tensor("bq2", [P, 2], F32, kind="ExternalInput").ap()
    out = nc.dram_tensor("out", [N, E], F32, kind="ExternalOutput").ap()

    with tile.TileContext(nc) as tc:
        with tc.tile_pool(name="persist", bufs=1) as pers, \
             tc.tile_pool(name="pexp", bufs=12) as pexp_pool, \
             tc.tile_pool(name="small", bufs=2) as small, \
             tc.tile_pool(name="ostage", bufs=6) as ostage, \
             tc.tile_pool(name="ppmain", bufs=1, space="PSUM") as ppm, \
             tc.tile_pool(name="ppfill", bufs=1, space="PSUM") as ppf, \
             tc.tile_pool(name="ppoacc", bufs=1, space="PSUM") as ppo:
            wq_sb = pers.tile([P, ECH, DG], F32R, tag="wq")
            wk_sb = pers.tile([P, ECH, DG], F32R, tag="wk")
            wv_sb = pers.tile([P, ECH, DG], F32R, tag="wv")
            wo_sb = pers.tile([P, 2, E], F32R, tag="wo")
            bq_sb = pers.tile([P, 2], F32, tag="bq")
            qT_p = [pers.tile([P, N], BF16, tag=f"qT{i}", name=f"qT{i}") for i in range(2)]
            # kTz[pair][h]: K^T of head h in its 64-row half, zeros in the
            # other half. S then uses a full 128-row stationary (the zero rows
            # annihilate the other head in the shared 128-row qT moving data);
            # partial-row stationaries defeat the PE's weight-load double
            # buffering and cost ~100ns exposed LDWEIGHTS per matmul.
            kTz_p = [[pers.tile([P, N], BF16, tag=f"kTz{i}{h}", name=f"kTz{i}{h}")
                      for h in range(2)] for i in range(2)]
            # v_sb[..., 0:64] = V, [..., 64:128] = ones. PV's stationary
            # [V | ones*64] is then a full [128,128] tile: PSUM rows 64:127
            # all come out equal to the softmax denominator (pre-broadcast),
            # so normalize is just reciprocal + multiply, no partition
            # broadcast, and the stationary is full-width for LD pipelining.
            v_sb = pers.tile([P, NCH, GROUPS, P], BF16, tag="v")
            oT_p = [pers.tile([P, N], F32R, tag=f"oT{i}", name=f"oT{i}") for i in range(2)]

            def fill_ps(i, name):
                return ppf.tile([P, QS], F32, tag="C" if i % 2 == 0 else "D", name=name)

            def k_chain(pair, qb):
                def emit():
                    ps = fill_ps(qb, f"kps{pair}{qb}")
                    for ec in range(ECH):
                        nc.tensor.matmul(
                            ps,
                            wk_sb[:, ec, ts(pair, P)],
                            _xt()[:, ec, ts(qb, QS)],
                            start=(ec == 0), stop=(ec == ECH - 1),
                        )
                    for h in range(2):
                        psl = slice(D * h, D * h + D)
                        nc.vector.tensor_copy(
                            kTz_p[pair][h][psl, ts(qb, QS)], ps[psl, :])
                return emit

            def q_chain(pair, qb):
                def emit():
                    ps = fill_ps(qb, f"qps{pair}{qb}")
                    for ec in range(ECH):
                        nc.tensor.matmul(
                            ps,
                            wq_sb[:, ec, ts(pair, P)],
                            _xt()[:, ec, ts(qb, QS)],
                            start=(ec == 0), stop=(ec == ECH - 1),
                        )
                    nc.vector.tensor_add(
                        qT_p[pair][:, ts(qb, QS)], ps,
                        bq_sb[:, pair, None].to_broadcast((P, QS)),
                    )
                return emit

            def v_chunk(ncx, pair):
                """V proj for one 128-row x chunk, one head pair (128 cols)."""
                def emit():
                    ps = fill_ps(ncx, f"vps{ncx}{pair}")
                    psl = ps[:, :P]
                    for ec in range(ECH):
                        nc.tensor.matmul(
                            psl,
                            _xt()[:, ec, ts(ncx, P)],
                            wv_sb[:, ec, ts(pair, P)],
                            start=(ec == 0), stop=(ec == ECH - 1),
                        )
                    nc.vector.tensor_copy(
                        v_sb[:, ncx, 2 * pair:2 * pair + 2, 0:64],
                        psl.rearrange("p (h d) -> p h d", d=D),
                    )
                return emit

            def wo_chain(ncx, fb):
                def emit():
                    ps = fill_ps(ncx * 2 + fb, f"wops{ncx}{fb}")
                    for dc in range(2):
                        nc.tensor.matmul(
                            ps,
                            oT_p[dc][:, ts(ncx, P)],
                            wo_sb[:, dc, ts(fb, QS)],
                            start=(dc == 0), stop=(dc == 1),
                        )
                    ot = ostage.tile([P, QS], F32, tag="ot", name="ot")
                    nc.vector.tensor_copy(ot, ps)
                    nc.sync.dma_start(out[ts(ncx, P), ts(fb, QS)], ot)
                return emit

            def emit_attn(pair, qs, fillers=()):
                """One [QS]-wide q block: S/EXP stream with PV lagged one kc;
                fillers run on their own PSUM banks between S and lagged PV."""
                fillers = list(fillers)
                step = max(1, NCH // (len(fillers) + 1)) if fillers else NCH + 1
                oaccs = [ppo.tile([P, QS], F32, tag=f"O{h}", name=f"oacc{h}")
                         for h in range(2)]
                prev_pes = None
                for kc in range(NCH + 1):
                    pes = None
                    if kc < NCH:
                        pes = []
                        for h in range(2):
                            ps = ppm.tile([P, QS], F32, tag=f"S{h}{kc % 2}",
                                          name=f"spsum{h}")
                            nc.tensor.matmul(
                                ps,
                                kTz_p[pair][h][:, ts(kc, P)],
                                qT_p[pair][:, ts(qs, QS)],
                                start=True, stop=True,
                            )
                            pe = pexp_pool.tile([P, QS], BF16, tag="pexp", name="pexp")
                            nc.scalar.activation(pe, ps, Exp, scale=1.0 / 32.0)
                            pes.append(pe)
                        if fillers and kc % step == 0:
                            fillers.pop(0)()
                    if prev_pes is not None:
                        kp = kc - 1
                        for h in range(2):
                            nc.tensor.matmul(
                                oaccs[h][:, :],
                                v_sb[:, kp, 2 * pair + h, :],
                                prev_pes[h],
                                start=(kp == 0), stop=(kp == NCH - 1),
                            )
                    prev_pes = pes
                for f in fillers:
                    f()
                # normalize: PSUM rows 64:128 hold the denominator (pre-
                # broadcast by the ones columns): oT = O'[0:64] * recip(denom)
                for h in range(2):
                    psl = slice(D * h, D * h + D)
                    den = small.tile([D, QS], F32, tag="den", name="den")
                    # reciprocal_approx_fast (custom DVE bit-trick) reads
                    # garbage from PSUM on HW — stage the denominators in SBUF
                    nc.vector.tensor_copy(den, oaccs[h][D:P, :])
                    rbc = small.tile([D, QS], F32, tag="rbc", name="rbc")
                    nc.vector.reciprocal_approx_fast(rbc, den)
                    nc.vector.tensor_mul(
                        oT_p[pair][psl, ts(qs, QS)],
                        oaccs[h][0:D, :],
                        rbc,
                    )

            with tc.tile_pool(name="xtp", bufs=1) as xtp:
                xt_sb = xtp.tile([P, ECH, N], F32R, tag="xt")
                _xt = lambda: xt_sb
                # DMA order matters: K chains start as soon as wk + early xt
                # chunks land; wo is not needed until much later.
                nc.sync.dma_start(wk_sb, wk.rearrange("(c p) d -> p c d", p=P))
                xt_r = xt.rearrange("(c p) n -> p c n", p=P)
                for ncx in range(NCH):
                    nc.sync.dma_start(xt_sb[:, :, ts(ncx, P)], xt_r[:, :, ts(ncx, P)])
                nc.sync.dma_start(wq_sb, wq.rearrange("(c p) d -> p c d", p=P))
                nc.sync.dma_start(bq_sb, bq2)
                nc.sync.dma_start(wv_sb, wv.rearrange("(c p) d -> p c d", p=P))
                nc.sync.dma_start(wo_sb, wo.rearrange("(c p) f -> p c f", p=P))
                for pair in range(2):
                    nc.gpsimd.memset(kTz_p[pair][0][D:, :], 0.0)
                    nc.gpsimd.memset(kTz_p[pair][1][0:D, :], 0.0)
                nc.gpsimd.memset(v_sb[:, :, :, D:P], 1.0)
                # upfront: all K pair 0, first Q block, pair-0 V chunks 0..3
                for qb in range(NQB):
                    k_chain(0, qb)()
                q_chain(0, 0)()
                for ncx in range(4):
                    v_chunk(ncx, 0)()
                emit_attn(0, 0, fillers=(
                    [v_chunk(ncx, 0) for ncx in range(4, NCH)] + [q_chain(0, 1)]
                ))
                emit_attn(0, 1, fillers=[q_chain(0, 2), k_chain(1, 0), k_chain(1, 1)])
                emit_attn(0, 2, fillers=[q_chain(0, 3), k_chain(1, 2), k_chain(1, 3)])
                emit_attn(0, 3, fillers=[q_chain(1, 0), q_chain(1, 1),
                                         v_chunk(0, 1), v_chunk(1, 1)])
                emit_attn(1, 0, fillers=(
                    [v_chunk(ncx, 1) for ncx in range(2, NCH)]
                    + [q_chain(1, 2), q_chain(1, 3)]
                ))
            emit_attn(1, 1, fillers=[wo_chain(ncx, fb) for ncx in range(0, 4)
                                     for fb in range(2)])
            emit_attn(1, 2, fillers=[wo_chain(ncx, fb) for ncx in range(4, 8)
                                     for fb in range(2)])
            emit_attn(1, 3, fillers=[wo_chain(ncx, fb) for ncx in range(8, 12)
                                     for fb in range(2)])
            for ncx in range(12, NCH):
                for fb in range(2):
                    wo_chain(ncx, fb)()

    nc.compile()
    return nc


def _get_nc():
    if "nc" not in _CACHE:
        _CACHE["nc"] = _build()
    return _CACHE["nc"]


def kernel(x, Wq, bq, Wk, bk, Wv, bv, Wo, bo, **run_kwargs):
    import sys
    if "/opt/trn_rl_repo" not in sys.path:
        sys.path.insert(0, "/opt/trn_rl_repo")
    from concourse.bass_utils import run_bass_kernel_spmd

    x = np.asarray(x, dtype=np.float32)
    Wq = np.asarray(Wq, dtype=np.float32)
    Wk = np.asarray(Wk, dtype=np.float32)
    Wv = np.asarray(Wv, dtype=np.float32)
    Wo = np.asarray(Wo, dtype=np.float32)
    bq = np.asarray(bq, dtype=np.float32)
    bv = np.asarray(bv, dtype=np.float32)
    bo = np.asarray(bo, dtype=np.float32)

    nc = _get_nc()

    in_maps = []
    xts = [_round_f32r(np.ascontiguousarray(x[b].T)) for b in range(B)]
    for c in range(NCORES):
        b, g = divmod(c, GROUPS)
        cols = slice(g * DG, (g + 1) * DG)
        in_maps.append({
            "xt": xts[b],
            "wq": _round_f32r(Wq[:, cols]),
            "wk": _round_f32r(Wk[:, cols]),
            "wv": _round_f32r(Wv[:, cols]),
            "wo": _round_f32r(Wo[cols, :]),
            "bq2": np.ascontiguousarray(bq[cols].reshape(2, P).T),
        })

    try:
        res = run_bass_kernel_spmd(nc, in_maps, core_ids=list(range(NCORES)), **run_kwargs)
    except Exception:
        # device may be wedged from a prior run; reset the accelerator once
        try:
            import ctypes
            lib = ctypes.CDLL("/opt/axon/libaxon_pjrt.so")
            lib.axon_reset.restype = ctypes.c_int
            lib.axon_reset()
        except Exception:
            pass
        res = run_bass_kernel_spmd(nc, in_maps, core_ids=list(range(NCORES)), **run_kwargs)
    if run_kwargs:
        _CACHE["last_results"] = res

    # gather: sum TP partials per batch, add separable bias terms
    bias_vec = bv @ Wo + bo  # softmax rows sum to 1 => bv contributes bv@Wo
    full = np.empty((B, N, E), dtype=np.float32)
    for b in range(B):
        acc = res.results[b * GROUPS]["out"].astype(np.float32).copy()
        for g in range(1, GROUPS):
            acc += res.results[b * GROUPS + g]["out"]
        full[b] = acc + bias_vec[None, :]
    return full


# revision 22
# speedup vs baseline: 1.4974x; 1.1279x over previous
"""Multi-head attention Trainium2 kernel (8 NeuronCores).

Problem: x[2,2048,1024] -> MHA(16 heads, d=64) -> out[2,2048,1024], fp32.

Sharding: 2-way data parallel on batch x 4-way tensor parallel on heads.
Core c handles batch c//4 and heads 4*(c%4) .. 4*(c%4)+3 (a 256-wide slice
of the Wq/Wk/Wv columns and Wo rows). Each core returns a partial output
[2048,1024]; the host sums the 4 TP partials per batch and adds the bias
terms (bo, and bv@Wo which is separable because softmax rows sum to 1;
bk drops out of softmax entirely since (q+bq)@bk is constant along keys).

On-core dataflow (projections fp32r, attention bf16):
  xt = x[b].T (host-transposed)      [1024, 2048]
  Q^T = Wq_g^T stationary over xt    [256, 2048]  (+bq, d on partitions)
  K^T likewise (no bias), V natural  [2048, 256]  via xt-stationary matmuls
  S^T[k,q] = K^T(d,k).T @ Q^T(d,q)   2 heads row-packed (d=64 each)
  P = exp(S^T / 32)                  ScalarE, scale fused, bf16 out
  O'^T[d+1,q] = [V|ones].T @ P       ones column gives softmax denominators
  O^T = O'[0:64] * (1/denom)         recip_approx_fast + gpsimd broadcast
  out = O^T.T @ Wo_g                 [2048, 1024] partial, DMA'd out

Scheduling: the TensorE runs DVFS p-states — it only sustains its fast
rate (~0.42 ns/row) while continuously busy; any bubble drops it to a
~1.3-2.5x slower state. ScalarE's exp() of the 4*2048*2048 score matrix
(~550ns per [128,512] tile) is longer per kc step than the S+PV matmuls,
so a naive schedule bubbles the PE every step and equilibrates at the
slow clock. To stay dense:
  - attention runs in [QS=512]-wide q blocks; per (kc, head) S is a
    single 512-free matmul into one of 4 parity-rotated single-bank PSUM
    tiles, so S(kc) never waits on EXP(kc-1) bank reads;
  - PV is emitted lagged one kc behind S/EXP, so its dependence on
    EXP(kc) is already satisfied when the PE reaches it — no stall;
  - two PSUM banks are reserved for filler chains (QK/V projections, Wo
    output matmuls), which are metered into every attention block
    between the S group and the lagged PV group to absorb the leftover
    per-kc PE slack and keep the clock pinned at the fast state.
"""

import numpy as np

B = 2
N = 2048
E = 1024
HEADS = 16
D = 64
P = 128
NCORES = 8
GROUPS = 4            # TP groups
DG = E // GROUPS      # 256 cols per core
ECH = E // P          # 8 contraction chunks
NCH = N // P          # 16 sequence chunks
QS = 512              # q span per attention block == matmul free dim
NQB = N // QS         # 4 q blocks per pair

_CACHE = {}


def _round_f32r(x: np.ndarray) -> np.ndarray:
    """Round fp32 to fp32r (e8m11): RNE on the low 12 mantissa bits."""
    u = np.ascontiguousarray(x, dtype=np.float32).view(np.uint32)
    lower = u & np.uint32(0xFFF)
    base = u & np.uint32(0xFFFFF000)
    up = (lower > np.uint32(1 << 11)) | (
        (lower == np.uint32(1 << 11)) & (((base >> np.uint32(12)) & np.uint32(1)) == 1)
    )
    return (base + np.where(up, np.uint32(1 << 12), np.uint32(0))).view(np.float32)


def _build():
    import sys
    if "/opt/trn_rl_repo" not in sys.path:
        sys.path.insert(0, "/opt/trn_rl_repo")
    import concourse.tile as tile
    from concourse import bacc, mybir
    from concourse.bass import ts

    F32 = mybir.dt.float32
    F32R = mybir.dt.float32r
    BF16 = mybir.dt.bfloat16
    Exp = mybir.ActivationFunctionType.Exp

    nc = bacc.Bacc("TRN2", target_bir_lowering=False, debug=False, num_devices=NCORES)

    xt = nc.dram_tensor("xt", [E, N], BF16, kind="ExternalInput").ap()
    wq = nc.dram_tensor("wq", [E, DG], BF16, kind="ExternalInput").ap()
    wk = nc.dram_tensor("wk", [E, DG], BF16, kind="ExternalInput").ap()
    wv = nc.dram_tensor("wv", [E, DG], BF16, kind="ExternalInput").ap()
    wo = nc.dram_tensor("wo", [DG, E], BF16, kind="ExternalInput").ap()
    bq2 = nc.dram_tensor("bq2", [P, 2], F32, kind="ExternalInput").ap()
    out = nc.dram_tensor("out", [N, E], F32, kind="ExternalOutput").ap()

    with tile.TileContext(nc) as tc:
        with tc.tile_pool(name="persist", bufs=1) as pers, \
             tc.tile_pool(name="pexp", bufs=12) as pexp_pool, \
             tc.tile_pool(name="small", bufs=2) as small, \
             tc.tile_pool(name="ostage", bufs=6) as ostage, \
             tc.tile_pool(name="ppmain", bufs=1, space="PSUM") as ppm, \
             tc.tile_pool(name="ppfill", bufs=1, space="PSUM") as ppf, \
             tc.tile_pool(name="ppoacc", bufs=1, space="PSUM") as ppo:
            wq_sb = pers.tile([P, ECH, DG], BF16, tag="wq")
            wk_sb = pers.tile([P, ECH, DG], BF16, tag="wk")
            wv_sb = pers.tile([P, ECH, DG], BF16, tag="wv")
            wo_sb = pers.tile([P, 2, E], BF16, tag="wo")
            bq_sb = pers.tile([P, 2], F32, tag="bq")
            qT_p = [pers.tile([P, N], BF16, tag=f"qT{i}", name=f"qT{i}") for i in range(2)]
            # kTz[pair][h]: K^T of head h in its 64-row half, zeros in the
            # other half. S then uses a full 128-row stationary (the zero rows
            # annihilate the other head in the shared 128-row qT moving data);
            # partial-row stationaries defeat the PE's weight-load double
            # buffering and cost ~100ns exposed LDWEIGHTS per matmul.
            kTz_p = [[pers.tile([P, N], BF16, tag=f"kTz{i}{h}", name=f"kTz{i}{h}")
                      for h in range(2)] for i in range(2)]
            # v_sb[..., 0:64] = V, [..., 64:128] = ones. PV's stationary
            # [V | ones*64] is then a full [128,128] tile: PSUM rows 64:127
            # all come out equal to the softmax denominator (pre-broadcast),
            # so normalize is just reciprocal + multiply, no partition
            # broadcast, and the stationary is full-width for LD pipelining.
            v_sb = pers.tile([P, NCH, GROUPS, P], BF16, tag="v")
            oT_p = [pers.tile([P, N], BF16, tag=f"oT{i}", name=f"oT{i}") for i in range(2)]

            def fill_ps(i, name):
                return ppf.tile([P, QS], F32, tag="C" if i % 2 == 0 else "D", name=name)

            def k_chain(pair, qb):
                def emit():
                    ps = fill_ps(qb, f"kps{pair}{qb}")
                    for ec in range(ECH):
                        nc.tensor.matmul(
                            ps,
                            wk_sb[:, ec, ts(pair, P)],
                            _xt()[:, ec, ts(qb, QS)],
                            start=(ec == 0), stop=(ec == ECH - 1),
                        )
                    for h in range(2):
                        psl = slice(D * h, D * h + D)
                        nc.vector.tensor_copy(
                            kTz_p[pair][h][psl, ts(qb, QS)], ps[psl, :])
                return emit

            def q_chain(pair, qb):
                def emit():
                    ps = fill_ps(qb, f"qps{pair}{qb}")
                    for ec in range(ECH):
                        nc.tensor.matmul(
                            ps,
                            wq_sb[:, ec, ts(pair, P)],
                            _xt()[:, ec, ts(qb, QS)],
                            start=(ec == 0), stop=(ec == ECH - 1),
                        )
                    nc.vector.tensor_add(
                        qT_p[pair][:, ts(qb, QS)], ps,
                        bq_sb[:, pair, None].to_broadcast((P, QS)),
                    )
                return emit

            def v_chunk(ncx, pair):
                """V proj for one 128-row x chunk, one head pair (128 cols)."""
                def emit():
                    ps = fill_ps(ncx, f"vps{ncx}{pair}")
                    psl = ps[:, :P]
                    for ec in range(ECH):
                        nc.tensor.matmul(
                            psl,
                            _xt()[:, ec, ts(ncx, P)],
                            wv_sb[:, ec, ts(pair, P)],
                            start=(ec == 0), stop=(ec == ECH - 1),
                        )
                    nc.vector.tensor_copy(
                        v_sb[:, ncx, 2 * pair:2 * pair + 2, 0:64],
                        psl.rearrange("p (h d) -> p h d", d=D),
                    )
                return emit

            def wo_chain(ncx, fb):
                def emit():
                    ps = fill_ps(ncx * 2 + fb, f"wops{ncx}{fb}")
                    for dc in range(2):
                        nc.tensor.matmul(
                            ps,
                            oT_p[dc][:, ts(ncx, P)],
                            wo_sb[:, dc, ts(fb, QS)],
                            start=(dc == 0), stop=(dc == 1),
                        )
                    ot = ostage.tile([P, QS], F32, tag="ot", name="ot")
                    nc.vector.tensor_copy(ot, ps)
                    nc.sync.dma_start(out[ts(ncx, P), ts(fb, QS)], ot)
                return emit

            def emit_attn(pair, qs, fillers=()):
                """One [QS]-wide q block: S/EXP stream with PV lagged one kc;
                fillers run on their own PSUM banks between S and lagged PV."""
                fillers = list(fillers)
                step = max(1, NCH // (len(fillers) + 1)) if fillers else NCH + 1
                oaccs = [ppo.tile([P, QS], F32, tag=f"O{h}", name=f"oacc{h}")
                         for h in range(2)]
                prev_pes = None
                for kc in range(NCH + 1):
                    pes = None
                    if kc < NCH:
                        pes = []
                        for h in range(2):
                            ps = ppm.tile([P, QS], F32, tag=f"S{h}{kc % 2}",
                                          name=f"spsum{h}")
                            nc.tensor.matmul(
                                ps,
                                kTz_p[pair][h][:, ts(kc, P)],
                                qT_p[pair][:, ts(qs, QS)],
                                start=True, stop=True,
                            )
                            pe = pexp_pool.tile([P, QS], BF16, tag="pexp", name="pexp")
                            nc.scalar.activation(pe, ps, Exp, scale=1.0 / 32.0)
                            pes.append(pe)
                        if fillers and kc % step == 0:
                            fillers.pop(0)()
                    if prev_pes is not None:
                        kp = kc - 1
                        for h in range(2):
                            nc.tensor.matmul(
                                oaccs[h][:, :],
                                v_sb[:, kp, 2 * pair + h, :],
                                prev_pes[h],
                                start=(kp == 0), stop=(kp == NCH - 1),
                            )
                    prev_pes = pes
                for f in fillers:
                    f()
                # normalize: PSUM rows 64:128 hold the denominator (pre-
                # broadcast by the ones columns): oT = O'[0:64] * recip(denom)
                for h in range(2):
                    psl = slice(D * h, D * h + D)
                    den = small.tile([D, QS], F32, tag="den", name="den")
                    # reciprocal_approx_fast (custom DVE bit-trick) reads
                    # garbage from PSUM on HW — stage the denominators in SBUF
                    nc.vector.tensor_copy(den, oaccs[h][D:P, :])
                    rbc = small.tile([D, QS], F32, tag="rbc", name="rbc")
                    nc.vector.reciprocal_approx_fast(rbc, den)
                    nc.vector.tensor_mul(
                        oT_p[pair][psl, ts(qs, QS)],
                        oaccs[h][0:D, :],
                        rbc,
                    )

            with tc.tile_pool(name="xtp", bufs=1) as xtp:
                xt_sb = xtp.tile([P, ECH, N], BF16, tag="xt")
                _xt = lambda: xt_sb
                # DMA order matters: K chains start as soon as wk + early xt
                # chunks land; wo is not needed until much later.
                nc.sync.dma_start(wk_sb, wk.rearrange("(c p) d -> p c d", p=P))
                xt_r = xt.rearrange("(c p) n -> p c n", p=P)
                for ncx in range(NCH):
                    nc.sync.dma_start(xt_sb[:, :, ts(ncx, P)], xt_r[:, :, ts(ncx, P)])
                nc.sync.dma_start(wq_sb, wq.rearrange("(c p) d -> p c d", p=P))
                nc.sync.dma_start(bq_sb, bq2)
                nc.sync.dma_start(wv_sb, wv.rearrange("(c p) d -> p c d", p=P))
                nc.sync.dma_start(wo_sb, wo.rearrange("(c p) f -> p c f", p=P))
                for pair in range(2):
                    nc.gpsimd.memset(kTz_p[pair][0][D:, :], 0.0)
                    nc.gpsimd.memset(kTz_p[pair][1][0:D, :], 0.0)
                nc.gpsimd.memset(v_sb[:, :, :, D:P], 1.0)
                # minimal upfront prefix; the rest of pair-0 K and V stream in
                # as early fillers of the first block (K(0,qb) is ready well
                # before S reaches kc=4*qb; V[j] before the lagged PV(j))
                k_chain(0, 0)()
                q_chain(0, 0)()
                for ncx in range(4):
                    v_chunk(ncx, 0)()
                emit_attn(0, 0, fillers=(
                    [k_chain(0, 1), k_chain(0, 2), k_chain(0, 3)]
                    + [v_chunk(ncx, 0) for ncx in range(4, NCH)] + [q_chain(0, 1)]
                ))
                emit_attn(0, 1, fillers=[q_chain(0, 2), k_chain(1, 0), k_chain(1, 1)])
                emit_attn(0, 2, fillers=[q_chain(0, 3), k_chain(1, 2), k_chain(1, 3)])
                emit_attn(0, 3, fillers=[q_chain(1, 0), q_chain(1, 1),
                                         v_chunk(0, 1), v_chunk(1, 1)])
                emit_attn(1, 0, fillers=(
                    [v_chunk(ncx, 1) for ncx in range(2, NCH)]
                    + [q_chain(1, 2), q_chain(1, 3)]
                ))
            emit_attn(1, 1, fillers=[wo_chain(ncx, fb) for ncx in range(0, 4)
                                     for fb in range(2)])
            emit_attn(1, 2, fillers=[wo_chain(ncx, fb) for ncx in range(4, 8)
                                     for fb in range(2)])
            emit_attn(1, 3, fillers=[wo_chain(ncx, fb) for ncx in range(8, 12)
                                     for fb in range(2)])
            for ncx in range(12, NCH):
                for fb in range(2):
                    wo_chain(ncx, fb)()

    nc.compile()
    return nc


def _get_nc():
    if "nc" not in _CACHE:
        _CACHE["nc"] = _build()
    return _CACHE["nc"]


def kernel(x, Wq, bq, Wk, bk, Wv, bv, Wo, bo, **run_kwargs):
    import sys
    if "/opt/trn_rl_repo" not in sys.path:
        sys.path.insert(0, "/opt/trn_rl_repo")
    from concourse.bass_utils import run_bass_kernel_spmd

    x = np.asarray(x, dtype=np.float32)
    Wq = np.asarray(Wq, dtype=np.float32)
    Wk = np.asarray(Wk, dtype=np.float32)
    Wv = np.asarray(Wv, dtype=np.float32)
    Wo = np.asarray(Wo, dtype=np.float32)
    bq = np.asarray(bq, dtype=np.float32)
    bv = np.asarray(bv, dtype=np.float32)
    bo = np.asarray(bo, dtype=np.float32)

    nc = _get_nc()

    import ml_dtypes
    bf16 = ml_dtypes.bfloat16
    in_maps = []
    xts = [np.ascontiguousarray(x[b].T).astype(bf16) for b in range(B)]
    for c in range(NCORES):
        b, g = divmod(c, GROUPS)
        cols = slice(g * DG, (g + 1) * DG)
        in_maps.append({
            "xt": xts[b],
            "wq": np.ascontiguousarray(Wq[:, cols]).astype(bf16),
            "wk": np.ascontiguousarray(Wk[:, cols]).astype(bf16),
            "wv": np.ascontiguousarray(Wv[:, cols]).astype(bf16),
            "wo": np.ascontiguousarray(Wo[cols, :]).astype(bf16),
            "bq2": np.ascontiguousarray(bq[cols].reshape(2, P).T),
        })

    try:
        res = run_bass_kernel_spmd(nc, in_maps, core_ids=list(range(NCORES)), **run_kwargs)
    except Exception:
        # device may be wedged from a prior run; reset the accelerator once
        try:
            import ctypes
            lib = ctypes.CDLL("/opt/axon/libaxon_pjrt.so")
            lib.axon_reset.restype = ctypes.c_int
            lib.axon_reset()
        except Exception:
            pass
        res = run_bass_kernel_spmd(nc, in_maps, core_ids=list(range(NCORES)), **run_kwargs)
    if run_kwargs:
        _CACHE["last_results"] = res

    # gather: sum TP partials per batch, add separable bias terms
    bias_vec = bv @ Wo + bo  # softmax rows sum to 1 => bv contributes bv@Wo
    full = np.empty((B, N, E), dtype=np.float32)
    for b in range(B):
        acc = res.results[b * GROUPS]["out"].astype(np.float32).copy()
        for g in range(1, GROUPS):
            acc += res.results[b * GROUPS + g]["out"]
        full[b] = acc + bias_vec[None, :]
    return full
